# revision 1
# baseline (speedup 1.0000x reference)
"""MoE grouped linear (gmm) kernel for 8 Trainium2 NeuronCores.

Strategy (expert parallel, mirrors the shard_map-over-gmm_sharded source):
  - Tokens arrive pre-sorted by expert; group_sizes[e] tokens belong to
    expert e. Core e gets weight[e] plus expert e's token slice, padded to
    MAXG rows so all 8 cores run one SPMD program. The "all-to-all" routing
    is host-side slicing, since kernel() sees the full inputs.
  - Per core we compute y_e^T = W_e^T @ X_e^T (out^T orientation): the
    weight tiles are the PE's stationary operand in natural [K, O] layout
    and X^T (prepared host-side) streams as the moving operand.
  - fp32 inputs are DMA'd untouched into resident SBUF tiles; the PE reads
    the high half of each fp32 word as bf16 through a bitcast + stride-2
    access pattern (truncation toward zero). The mean truncation shrink is
    measured host-side and compensated via the ScalarE evacuation scale;
    the per-partition bias is fused into the same instruction. PSUM
    accumulates in fp32.
Host then unpads/concatenates per-expert outputs back to [T, Out] fp32.
"""

import numpy as np

import concourse.bass as bass
from concourse import bacc
import concourse.mybir as mybir
import concourse.tile as tile
from concourse.bass_utils import run_bass_kernel_spmd

N_CORES = 8
P = 128

_BUILD_CACHE: dict = {}


def _t_chunks(maxg: int) -> list[tuple[int, int]]:
    """Split the token free-dim into PSUM-bank-sized (<=512) chunks."""
    n = (maxg + 511) // 512
    base = ((maxg // n + P - 1) // P) * P
    chunks = []
    off = 0
    while off < maxg:
        sz = min(base, maxg - off)
        chunks.append((off, sz))
        off += sz
    return chunks


def _build_program(maxg: int, n_in: int, n_out: int):
    kb = n_in // P   # contraction blocks
    ob = n_out // P  # output-row blocks
    f32 = mybir.dt.float32
    bf16 = mybir.dt.bfloat16

    nc = bacc.Bacc(
        "TRN2", target_bir_lowering=False, debug=False, num_devices=N_CORES
    )
    xt = nc.dram_tensor("xt", [n_in, maxg], f32, kind="ExternalInput")
    # W pre-tiled host-side: [ob, P(partition of k-block), kb, P(o)] so each
    # o-slab DMA is fully contiguous per partition (8 KiB segments).
    w = nc.dram_tensor("w", [ob, P, kb, P], f32, kind="ExternalInput")
    bias = nc.dram_tensor("bias", [P, ob], f32, kind="ExternalInput")
    sc = nc.dram_tensor("sc", [P, 2], f32, kind="ExternalInput")
    yt = nc.dram_tensor("yt", [n_out, maxg], f32, kind="ExternalOutput")

    chunks = _t_chunks(maxg)

    # o-blocks processed concurrently in group 0; 8 PSUM banks available.
    GRP = max(1, min(4, 8 // len(chunks), ob))
    # k-slabs per X quarter-tile.
    XQ = next(q for q in (4, 2, 1) if kb % q == 0)

    with tile.TileContext(nc) as tc:
        with (
            tc.tile_pool(name="const", bufs=1) as constp,
            tc.tile_pool(name="xtsb", bufs=1) as xtp,
            tc.tile_pool(name="wsb", bufs=2 * GRP) as wp,
            tc.tile_pool(name="outsb", bufs=2 * GRP) as outp,
            tc.tile_pool(name="wbsb", bufs=4) as wbp,
            tc.tile_pool(name="psum", bufs=1, space="PSUM") as psump,
        ):
            bias_sb = constp.tile([P, ob], f32)
            nc.scalar.dma_start(bias_sb[:], bias[:])
            sc_sb = constp.tile([P, 2], f32)
            nc.scalar.dma_start(sc_sb[:], sc[:])

            def load_w(o):
                w_o = wp.tile([P, kb, P], f32, tag="wo", name=f"w{o}")
                nc.sync.dma_start(w_o[:], w[o])
                return w_o

            def load_xq(q):
                xq = xtp.tile([P, XQ, maxg], f32, tag=f"xq{q}", name=f"xq{q}")
                nc.sync.dma_start(
                    xq[:],
                    xt[q * XQ * P : (q + 1) * XQ * P, :].rearrange(
                        "(k p) t -> p k t", p=P
                    ),
                )
                return xq[:].bitcast(bf16).rearrange(
                    "p k (t two) -> p k t two", two=2
                )

            # Interleave the prologue DMAs: the single HW ring delivers in
            # order, and group 0 needs w0..w3 plus all of X before its end.
            nq = kb // XQ
            prologue = []
            wi = xi = 0
            while wi < GRP or xi < nq:
                if wi < GRP:
                    prologue.append(("w", wi)); wi += 1
                if xi < nq:
                    prologue.append(("x", xi)); xi += 1
            w_pref = {}
            xqs = [None] * nq
            for kind, i in prologue:
                if kind == "w":
                    w_pref[i] = load_w(i)
                else:
                    xqs[i] = load_xq(i)

            # HAM warmup: dummy matmuls with no data deps run while the
            # prologue DMAs stream, so the PE clock is at 2.4 GHz (and the
            # activity window warm) when the first real matmul issues.
            warm = constp.tile([P, 512], bf16)
            nc.gpsimd.memset(warm[:], 0)
            ps_warm = psump.tile([P, 512], f32, tag="ps0_0", name="warmps")
            for i in range(24):
                nc.tensor.matmul(
                    ps_warm[:],
                    warm[:, :P],
                    warm[:],
                    start=(i == 0),
                    stop=(i == 23),
                )

            def evac(ps, o, t0, tsz, engine, sci=0):
                """PSUM -> SBUF with fused scale + per-o bias, then store."""
                ot = outp.tile([P, tsz], f32, tag="ot", name=f"ot{o}_{t0}")
                if engine == 0:
                    nc.scalar.activation(
                        ot[:],
                        ps[:],
                        mybir.ActivationFunctionType.Identity,
                        bias=bias_sb[:, o : o + 1],
                        scale=sc_sb[:, sci : sci + 1],
                    )
                else:
                    nc.vector.tensor_scalar(
                        ot[:],
                        ps[:],
                        sc_sb[:, sci : sci + 1],
                        bias_sb[:, o : o + 1],
                        mybir.AluOpType.mult,
                        mybir.AluOpType.add,
                    )
                nc.scalar.dma_start(yt[o * P : (o + 1) * P, t0 : t0 + tsz], ot[:])

            # Group 0 (o-blocks 0..GRP-1) runs k-major so every arriving
            # X-slab immediately feeds GRP o-columns of PE work; it owns all
            # 2*GRP PSUM banks.
            g0 = list(range(GRP))
            wovs0 = [
                w_pref.pop(o)[:]
                .bitcast(bf16)
                .rearrange("p k (o two) -> p k o two", two=2)
                for o in g0
            ]
            pss0 = {
                (oi, ti): psump.tile(
                    [P, tsz], f32, tag=f"ps{oi}_{ti}", name=f"ps{oi}_{ti}"
                )
                for oi in g0
                for ti, (t0, tsz) in enumerate(chunks)
            }
            for k in range(kb):
                xvk = xqs[k // XQ]
                for oi in g0:
                    for ti, (t0, tsz) in enumerate(chunks):
                        nc.tensor.matmul(
                            pss0[oi, ti][:],
                            wovs0[oi][:, k, :, 1],
                            xvk[:, k % XQ, t0 : t0 + tsz, 1],
                            start=(k == 0),
                            stop=(k == kb - 1),
                        )
            for oi in g0:
                for ti, (t0, tsz) in enumerate(chunks):
                    evac(pss0[oi, ti], oi, t0, tsz, (oi + ti) % 2)

            # Remaining o-blocks run one at a time: per-bank k-runs rotate
            # through the PSUM banks (released by group 0 in the same
            # order), and evacuations pipeline under the next bank's MMs.
            for o in range(GRP, ob):
                w_o = w_pref.pop(o) if o in w_pref else load_w(o)
                wb = wbp.tile([P, kb, P], bf16, tag="wb", name=f"wb{o}")
                nc.vector.tensor_copy(wb[:], w_o[:])
                for ti, (t0, tsz) in enumerate(chunks):
                    ps = psump.tile(
                        [P, tsz],
                        f32,
                        tag=f"ps{o % GRP}_{ti}",
                        name=f"ps{o}_{ti}",
                    )
                    for k in range(kb):
                        nc.tensor.matmul(
                            ps[:],
                            wb[:, k, :],
                            xqs[k // XQ][:, k % XQ, t0 : t0 + tsz, 1],
                            start=(k == 0),
                            stop=(k == kb - 1),
                        )
                    evac(ps, o, t0, tsz, (o + ti) % 2, sci=1)
    nc.finalize()
    return nc


def _trunc_ratio(a: np.ndarray) -> float:
    """mean(|trunc_bf16(a)|) / mean(|a|) — the systematic shrink from
    reading only the high 16 bits of each fp32."""
    t = (a.view(np.uint32) & np.uint32(0xFFFF0000)).view(np.float32)
    denom = float(np.abs(a).sum())
    if denom == 0.0:
        return 1.0
    return float(np.abs(t).sum()) / denom


def _prepare(inputs, weight, bias, group_sizes):
    """Build (or reuse) the program and the per-core input maps."""
    inputs = np.ascontiguousarray(np.asarray(inputs, dtype=np.float32))
    weight = np.ascontiguousarray(np.asarray(weight, dtype=np.float32))
    bias = np.ascontiguousarray(np.asarray(bias, dtype=np.float32))
    g = np.asarray(group_sizes).astype(np.int64)

    t_tokens, n_in = inputs.shape
    n_exp, _, n_out = weight.shape
    assert n_exp == N_CORES, f"expected {N_CORES} experts, got {n_exp}"
    offs = np.concatenate([[0], np.cumsum(g)])
    assert offs[-1] == t_tokens, "group_sizes must sum to token count"

    maxg = max(P, int(-(-int(g.max()) // P)) * P)

    key = (maxg, n_in, n_out)
    if key not in _BUILD_CACHE:
        _BUILD_CACHE[key] = _build_program(maxg, n_in, n_out)
    nc = _BUILD_CACHE[key]

    ob = n_out // P
    bias_host = np.ascontiguousarray(bias.reshape(ob, P).T)  # [P, ob]

    # Compensate the mean truncation shrink: col 0 for truncated X and W
    # (group 0), col 1 for truncated X with round-to-nearest W (later os).
    rx, rw = _trunc_ratio(inputs), _trunc_ratio(weight)
    sc_host = np.empty((P, 2), np.float32)
    sc_host[:, 0] = 1.0 / (rx * rw)
    sc_host[:, 1] = 1.0 / rx

    in_maps = []
    for e in range(n_exp):
        xe = inputs[offs[e] : offs[e + 1]]  # [g_e, n_in]
        xt_e = np.zeros((n_in, maxg), np.float32)
        xt_e[:, : g[e]] = xe.T
        w_e = np.ascontiguousarray(
            weight[e].reshape(kb := n_in // P, P, ob, P).transpose(2, 1, 0, 3)
        )  # [ob, P(k within block), kb, P(o)]
        in_maps.append(
            {"xt": xt_e, "w": w_e, "bias": bias_host, "sc": sc_host}
        )
    return nc, in_maps, g, offs, (t_tokens, n_out)


def kernel(inputs, weight, bias, group_sizes):
    nc, in_maps, g, offs, (t_tokens, n_out) = _prepare(
        inputs, weight, bias, group_sizes
    )
    res = run_bass_kernel_spmd(nc, in_maps, core_ids=list(range(N_CORES)))

    out = np.empty((t_tokens, n_out), np.float32)
    for e in range(N_CORES):
        if g[e] == 0:
            continue
        yt_e = res.results[e]["yt"]  # [n_out, maxg]
        out[offs[e] : offs[e + 1]] = yt_e[:, : g[e]].T
    return out



# revision 2
# speedup vs baseline: 1.3344x; 1.3344x over previous
"""MoE grouped linear (gmm) kernel for 8 Trainium2 NeuronCores.

Strategy (expert parallel + token load balancing, bf16 compute):
  - Tokens arrive pre-sorted by expert; group_sizes[e] tokens belong to
    expert e. Core e gets weight[e] plus up to C=512 of expert e's tokens
    (the balanced share, T/8). The "all-to-all" routing is host-side
    slicing, since kernel() sees the full inputs.
  - Excess tokens of heavy experts (g_e > C) are split into guest items of
    (<=128 tokens x one half of the output dim) and scattered one per core,
    so every core does the same 512+64-column-equivalent of PE work instead
    of padding everyone to max(g_e). Guest outputs are final values for
    their (token, out) rectangle - no cross-core reduction.
  - X and W are converted to bf16 (round-to-nearest) host-side, halving
    HBM->SBUF traffic; the PE computes bf16 x bf16 -> fp32 PSUM. Per-core
    DMA (~15.7 MB, ~44 us) then hides fully under PE time (~61 us).
  - Per core we compute y^T = W^T @ X^T (out^T orientation): weight tiles
    are the PE's stationary operand in natural [K, O] layout and X^T
    (prepared host-side) streams as the moving operand. The per-partition
    bias is fused into the PSUM evacuation instruction.
Host then scatters per-core main/guest outputs back to [T, Out] fp32.
"""

import numpy as np
import ml_dtypes

import concourse.bass as bass
from concourse import bacc
import concourse.mybir as mybir
import concourse.tile as tile
from concourse.bass_utils import run_bass_kernel_spmd

N_CORES = 8
P = 128
GT = 128  # guest item token width

_BUILD_CACHE: dict = {}


def _build_program(c_main: int, n_in: int, n_out: int, s_guest: int):
    kb = n_in // P   # contraction blocks
    ob = n_out // P  # output-row blocks
    oh = ob // 2     # guest o-blocks (half the output dim)
    f32 = mybir.dt.float32
    bf16 = mybir.dt.bfloat16

    nc = bacc.Bacc(
        "TRN2", target_bir_lowering=False, debug=False, num_devices=N_CORES
    )
    # X^T pre-tiled host-side: [P(k within block), kb, C] so each quarter
    # DMA is fully contiguous per partition.
    xm = nc.dram_tensor("xm", [P, kb, c_main], bf16, kind="ExternalInput")
    # W pre-tiled host-side: [ob, P(partition of k-block), kb, P(o)] so each
    # o-slab DMA is fully contiguous per partition (4 KiB segments).
    wm = nc.dram_tensor("wm", [ob, P, kb, P], bf16, kind="ExternalInput")
    biasm = nc.dram_tensor("biasm", [P, ob], f32, kind="ExternalInput")
    ym = nc.dram_tensor("ym", [n_out, c_main], f32, kind="ExternalOutput")
    if s_guest:
        xg = nc.dram_tensor("xg", [s_guest, P, kb, GT], bf16, kind="ExternalInput")
        wg = nc.dram_tensor(
            "wg", [s_guest, oh, P, kb, P], bf16, kind="ExternalInput"
        )
        biasg = nc.dram_tensor("biasg", [s_guest, P, oh], f32, kind="ExternalInput")
        yg = nc.dram_tensor(
            "yg", [s_guest, oh * P, GT], f32, kind="ExternalOutput"
        )

    # o-blocks processed concurrently in group 0 (k-major); 8 PSUM banks.
    GRP = 4
    XQ = 4  # k-slabs per X quarter-tile

    with tile.TileContext(nc) as tc:
        with (
            tc.tile_pool(name="const", bufs=1) as constp,
            tc.tile_pool(name="xtsb", bufs=1) as xtp,
            tc.tile_pool(name="wsb", bufs=ob) as wp,
            tc.tile_pool(name="wgsb", bufs=max(1, min(8, s_guest * oh))) as wgp,
            tc.tile_pool(name="outsb", bufs=2 * GRP) as outp,
            tc.tile_pool(name="goutsb", bufs=4) as goutp,
            tc.tile_pool(name="psum", bufs=1, space="PSUM") as psump,
        ):
            bias_sb = constp.tile([P, ob], f32)
            nc.scalar.dma_start(bias_sb[:], biasm[:])
            if s_guest:
                biasg_sb = constp.tile([P, s_guest * oh], f32)
                nc.scalar.dma_start(
                    biasg_sb[:], biasg.rearrange("s p o -> p (s o)")
                )

            def load_w(o):
                w_o = wp.tile([P, kb, P], bf16, tag="wo", name=f"w{o}")
                nc.sync.dma_start(w_o[:], wm[o])
                return w_o

            def load_xq(q):
                xq = xtp.tile([P, XQ, c_main], bf16, tag=f"xq{q}", name=f"xq{q}")
                nc.sync.dma_start(xq[:], xm[:, q * XQ : (q + 1) * XQ, :])
                return xq

            # Interleave the prologue DMAs: the HW ring delivers in order,
            # and group 0 needs w0..w3 plus all of X before its end.
            nq = kb // XQ
            prologue = []
            wi = xi = 0
            while wi < GRP or xi < nq:
                if wi < GRP:
                    prologue.append(("w", wi)); wi += 1
                if xi < nq:
                    prologue.append(("x", xi)); xi += 1
            w_pref = {}
            xqs = [None] * nq
            for kind, i in prologue:
                if kind == "w":
                    w_pref[i] = load_w(i)
                else:
                    xqs[i] = load_xq(i)
            # Guest X streams right after the main prologue (tiny), guest
            # weights after it; both are consumed only in the guest pass.
            if s_guest:
                xg_sb = []
                for s in range(s_guest):
                    t = xtp.tile([P, kb, GT], bf16, tag=f"xg{s}", name=f"xg{s}")
                    nc.sync.dma_start(t[:], xg[s])
                    xg_sb.append(t)

            # HAM warmup: dummy matmuls with no data deps run while the
            # prologue DMAs stream, so the PE clock is at 2.4 GHz (and the
            # activity window warm) when the first real matmul issues.
            warm = constp.tile([P, 512], bf16)
            nc.gpsimd.memset(warm[:], 0)
            ps_warm = psump.tile([P, 512], f32, tag="ps0", name="warmps")
            for i in range(8):
                nc.tensor.matmul(
                    ps_warm[:],
                    warm[:, :P],
                    warm[:],
                    start=(i == 0),
                    stop=(i == 7),
                )

            def evac(ps, bias_col, dst, n_cols, engine, name):
                """PSUM -> SBUF with fused per-o bias, then store."""
                pool = outp if n_cols == c_main else goutp
                ot = pool.tile([P, n_cols], f32, tag=f"ot{n_cols}", name=name)
                if engine == 0:
                    nc.scalar.activation(
                        ot[:],
                        ps[:],
                        mybir.ActivationFunctionType.Identity,
                        bias=bias_col,
                    )
                else:
                    nc.vector.tensor_scalar_add(ot[:], ps[:], bias_col)
                nc.scalar.dma_start(dst, ot[:])

            # Group 0 (o-blocks 0..GRP-1) runs k-major so every arriving
            # X-slab immediately feeds GRP o-columns of PE work.
            g0 = list(range(GRP))
            pss0 = {
                oi: psump.tile([P, c_main], f32, tag=f"ps{oi}", name=f"ps{oi}")
                for oi in g0
            }
            for k in range(kb):
                xvk = xqs[k // XQ]
                for oi in g0:
                    nc.tensor.matmul(
                        pss0[oi][:],
                        w_pref[oi][:, k, :],
                        xvk[:, k % XQ, :],
                        start=(k == 0),
                        stop=(k == kb - 1),
                    )
            for oi in g0:
                evac(
                    pss0[oi], bias_sb[:, oi : oi + 1],
                    ym[oi * P : (oi + 1) * P, :], c_main, oi % 2, f"ot{oi}",
                )

            # Remaining o-blocks run one at a time: per-bank k-runs rotate
            # through the PSUM banks (released by group 0 in the same
            # order), and evacuations pipeline under the next bank's MMs.
            for o in range(GRP, ob):
                w_o = load_w(o)
                ps = psump.tile([P, c_main], f32, tag=f"ps{o % GRP}", name=f"ps{o}")
                for k in range(kb):
                    nc.tensor.matmul(
                        ps[:],
                        w_o[:, k, :],
                        xqs[k // XQ][:, k % XQ, :],
                        start=(k == 0),
                        stop=(k == kb - 1),
                    )
                evac(
                    ps, bias_sb[:, o : o + 1],
                    ym[o * P : (o + 1) * P, :], c_main, o % 2, f"ot{o}",
                )

            # Guest pass: s_guest items of (GT tokens x oh o-blocks), final
            # outputs for their rectangle (no cross-core reduction).
            if s_guest:
                for s in range(s_guest):
                    for o in range(oh):
                        wg_o = wgp.tile(
                            [P, kb, P], bf16, tag="wg", name=f"wg{s}_{o}"
                        )
                        nc.sync.dma_start(wg_o[:], wg[s, o])
                        ps = psump.tile(
                            [P, GT], f32, tag=f"psg{o % 2}", name=f"psg{s}_{o}"
                        )
                        for k in range(kb):
                            nc.tensor.matmul(
                                ps[:],
                                wg_o[:, k, :],
                                xg_sb[s][:, k, :],
                                start=(k == 0),
                                stop=(k == kb - 1),
                            )
                        evac(
                            ps, biasg_sb[:, s * oh + o : s * oh + o + 1],
                            yg[s, o * P : (o + 1) * P, :], GT, o % 2,
                            f"go{s}_{o}",
                        )
    nc.finalize()
    return nc


def _tile_x(x_cols: np.ndarray, n_in: int, width: int) -> np.ndarray:
    """[n, n_in] fp32 token rows -> [P, kb, width] bf16 X^T tiling."""
    kb = n_in // P
    xt = np.zeros((n_in, width), np.float32)
    xt[:, : x_cols.shape[0]] = x_cols.T
    return np.ascontiguousarray(
        xt.reshape(kb, P, width).transpose(1, 0, 2)
    ).astype(ml_dtypes.bfloat16)


def _tile_w(w: np.ndarray) -> np.ndarray:
    """[n_in, n_o] fp32 -> [n_o//P, P, n_in//P, P] bf16 o-slab tiling."""
    n_in, n_o = w.shape
    return np.ascontiguousarray(
        w.reshape(n_in // P, P, n_o // P, P).transpose(2, 1, 0, 3)
    ).astype(ml_dtypes.bfloat16)


def _prepare(inputs, weight, bias, group_sizes):
    """Build (or reuse) the program and the per-core input maps."""
    inputs = np.ascontiguousarray(np.asarray(inputs, dtype=np.float32))
    weight = np.ascontiguousarray(np.asarray(weight, dtype=np.float32))
    bias = np.ascontiguousarray(np.asarray(bias, dtype=np.float32))
    g = np.asarray(group_sizes).astype(np.int64)

    t_tokens, n_in = inputs.shape
    n_exp, _, n_out = weight.shape
    assert n_exp == N_CORES, f"expected {N_CORES} experts, got {n_exp}"
    offs = np.concatenate([[0], np.cumsum(g)])
    assert offs[-1] == t_tokens, "group_sizes must sum to token count"

    kb, ob, oh = n_in // P, n_out // P, n_out // P // 2
    c_main = max(P, -(-t_tokens // N_CORES // P) * P)  # balanced share

    # Guest items: (expert, token-block, out-half) for tokens beyond c_main.
    items = []
    for e in range(n_exp):
        nblk = -(-max(0, int(g[e]) - c_main) // GT)
        for b in range(nblk):
            for h in range(2):
                items.append((e, b, h))
    s_guest = -(-len(items) // N_CORES) if items else 0

    key = (c_main, n_in, n_out, s_guest)
    if key not in _BUILD_CACHE:
        _BUILD_CACHE[key] = _build_program(c_main, n_in, n_out, s_guest)
    nc = _BUILD_CACHE[key]

    biasm_host = np.ascontiguousarray(bias.reshape(ob, P).T)  # [P, ob]

    # slot-major assignment: item i -> (core i % 8, slot i // 8)
    plan = [[None] * s_guest for _ in range(n_exp)]
    for i, it in enumerate(items):
        plan[i % N_CORES][i // N_CORES] = it

    in_maps = []
    for e in range(n_exp):
        nm = min(int(g[e]), c_main)
        m = {
            "xm": _tile_x(inputs[offs[e] : offs[e] + nm], n_in, c_main),
            "wm": _tile_w(weight[e]),
            "biasm": biasm_host,
        }
        if s_guest:
            xg_l, wg_l, bg_l = [], [], []
            for it in plan[e]:
                if it is None:
                    xg_l.append(np.zeros((P, kb, GT), ml_dtypes.bfloat16))
                    wg_l.append(np.zeros((oh, P, kb, P), ml_dtypes.bfloat16))
                    bg_l.append(np.zeros((P, oh), np.float32))
                else:
                    ge, b, h = it
                    t0 = offs[ge] + c_main + b * GT
                    n = min(GT, offs[ge + 1] - t0)
                    xg_l.append(_tile_x(inputs[t0 : t0 + n], n_in, GT))
                    wg_l.append(
                        _tile_w(weight[ge][:, h * oh * P : (h + 1) * oh * P])
                    )
                    bg_l.append(
                        np.ascontiguousarray(
                            bias.reshape(ob, P)[h * oh : (h + 1) * oh].T
                        )
                    )
            m["xg"] = np.stack(xg_l)
            m["wg"] = np.stack(wg_l)
            m["biasg"] = np.stack(bg_l)
        in_maps.append(m)
    meta = (g, offs, plan, c_main, s_guest)
    return nc, in_maps, meta, None, (t_tokens, n_out)


def kernel(inputs, weight, bias, group_sizes):
    nc, in_maps, meta, _, (t_tokens, n_out) = _prepare(
        inputs, weight, bias, group_sizes
    )
    g, offs, plan, c_main, s_guest = meta
    oh = n_out // P // 2
    res = run_bass_kernel_spmd(nc, in_maps, core_ids=list(range(N_CORES)))

    out = np.empty((t_tokens, n_out), np.float32)
    for e in range(N_CORES):
        nm = min(int(g[e]), c_main)
        if nm:
            out[offs[e] : offs[e] + nm] = res.results[e]["ym"][:, :nm].T
        for s in range(s_guest):
            it = plan[e][s]
            if it is None:
                continue
            ge, b, h = it
            t0 = offs[ge] + c_main + b * GT
            n = min(GT, int(offs[ge + 1] - t0))
            out[t0 : t0 + n, h * oh * P : (h + 1) * oh * P] = res.results[e][
                "yg"
            ][s][:, :n].T
    return out


# revision 4
# speedup vs baseline: 1.3734x; 1.0292x over previous
"""MoE grouped linear (gmm) kernel for 8 Trainium2 NeuronCores.

Strategy (expert parallel + token load balancing, bf16 compute):
  - Tokens arrive pre-sorted by expert; group_sizes[e] tokens belong to
    expert e. Core e gets weight[e] plus up to C=512 of expert e's tokens
    (the balanced share, T/8). The "all-to-all" routing is host-side
    slicing, since kernel() sees the full inputs.
  - Excess tokens of heavy experts (g_e > C) are split into guest items of
    (<=128 tokens x one half of the output dim) and scattered one per core,
    so every core does the same 512+64-column-equivalent of PE work instead
    of padding everyone to max(g_e). Guest outputs are final values for
    their (token, out) rectangle - no cross-core reduction.
  - X and W are converted to bf16 (round-to-nearest) host-side, halving
    HBM->SBUF traffic; the PE computes bf16 x bf16 -> fp32 PSUM. Per-core
    DMA (~15.7 MB, ~44 us) then hides fully under PE time (~61 us).
  - Per core we compute y^T = W^T @ X^T (out^T orientation): weight tiles
    are the PE's stationary operand in natural [K, O] layout and X^T
    (prepared host-side) streams as the moving operand. The per-partition
    bias is fused into the PSUM evacuation instruction.
Host then scatters per-core main/guest outputs back to [T, Out] fp32.
"""

import numpy as np
import ml_dtypes

import concourse.bass as bass
from concourse import bacc
import concourse.mybir as mybir
import concourse.tile as tile
from concourse.bass_utils import run_bass_kernel_spmd

N_CORES = 8
P = 128
GT = 128  # guest item token width

_BUILD_CACHE: dict = {}


def _build_program(c_main: int, n_in: int, n_out: int, s_guest: int):
    kb = n_in // P   # contraction blocks
    ob = n_out // P  # output-row blocks
    oh = ob // 2     # guest o-blocks (half the output dim)
    f32 = mybir.dt.float32
    bf16 = mybir.dt.bfloat16

    nc = bacc.Bacc(
        "TRN2", target_bir_lowering=False, debug=False, num_devices=N_CORES
    )
    # X^T pre-tiled host-side: [P(k within block), kb, C] so each quarter
    # DMA is fully contiguous per partition.
    xm = nc.dram_tensor("xm", [P, kb, c_main], bf16, kind="ExternalInput")
    # W pre-tiled host-side: [ob, P(partition of k-block), kb, P(o)] so each
    # o-slab DMA is fully contiguous per partition (4 KiB segments).
    wm = nc.dram_tensor("wm", [ob, P, kb, P], bf16, kind="ExternalInput")
    biasm = nc.dram_tensor("biasm", [P, ob], f32, kind="ExternalInput")
    ym = nc.dram_tensor("ym", [n_out, c_main], f32, kind="ExternalOutput")
    if s_guest:
        xg = nc.dram_tensor("xg", [s_guest, P, kb, GT], bf16, kind="ExternalInput")
        wg = nc.dram_tensor(
            "wg", [s_guest, oh, P, kb, P], bf16, kind="ExternalInput"
        )
        biasg = nc.dram_tensor("biasg", [s_guest, P, oh], f32, kind="ExternalInput")
        yg = nc.dram_tensor(
            "yg", [s_guest, oh * P, GT], f32, kind="ExternalOutput"
        )

    # o-blocks processed concurrently in group 0 (k-major); 8 PSUM banks.
    GRP = 4
    XQ = 4  # k-slabs per X quarter-tile

    with tile.TileContext(nc) as tc:
        with (
            tc.tile_pool(name="const", bufs=1) as constp,
            tc.tile_pool(name="xtsb", bufs=1) as xtp,
            tc.tile_pool(name="wsb", bufs=ob) as wp,
            tc.tile_pool(name="wgsb", bufs=max(1, min(8, s_guest * oh))) as wgp,
            tc.tile_pool(name="outsb", bufs=2 * GRP) as outp,
            tc.tile_pool(name="goutsb", bufs=4) as goutp,
            tc.tile_pool(name="psum", bufs=1, space="PSUM") as psump,
        ):
            def load_w(o, engine=None):
                w_o = wp.tile([P, kb, P], bf16, tag="wo", name=f"w{o}")
                (engine or nc.sync).dma_start(w_o[:], wm[o])
                return w_o

            def load_xq(q):
                xq = xtp.tile([P, XQ, c_main], bf16, tag=f"xq{q}", name=f"xq{q}")
                nc.sync.dma_start(xq[:], xm[:, q * XQ : (q + 1) * XQ, :])
                return xq

            # Prologue DMAs ride BOTH HW-DGE rings so w0 and xq0 transfer
            # in parallel: scalar ring gets w0 (+ the tiny consts), sync
            # ring gets the X quarters interleaved with w1..w3. Each ring
            # delivers in order.
            w_pref = {0: load_w(0, nc.scalar)}
            bias_sb = constp.tile([P, ob], f32)
            nc.scalar.dma_start(bias_sb[:], biasm[:])
            if s_guest:
                biasg_sb = constp.tile([P, s_guest * oh], f32)
                nc.scalar.dma_start(
                    biasg_sb[:], biasg.rearrange("s p o -> p (s o)")
                )
            nq = kb // XQ
            xqs = [None] * nq
            for i in range(nq):
                xqs[i] = load_xq(i)
                if i + 1 < GRP:
                    w_pref[i + 1] = load_w(i + 1)
            # Guest X streams right after the main prologue (tiny), guest
            # weights after it; both are consumed only in the guest pass.
            if s_guest:
                xg_sb = []
                for s in range(s_guest):
                    t = xtp.tile([P, kb, GT], bf16, tag=f"xg{s}", name=f"xg{s}")
                    nc.scalar.dma_start(t[:], xg[s])
                    xg_sb.append(t)

            # HAM warmup: dummy matmuls with no data deps run while the
            # prologue DMAs stream, so the PE clock is at 2.4 GHz (and the
            # activity window warm) when the first real matmul issues.
            # DVE memset: the gpsimd path boots ~2us slower.
            warm = constp.tile([P, 512], bf16)
            nc.vector.memset(warm[:], 0)
            ps_warm = psump.tile([P, 512], f32, tag="ps0", name="warmps")
            for i in range(8):
                nc.tensor.matmul(
                    ps_warm[:],
                    warm[:, :P],
                    warm[:],
                    start=(i == 0),
                    stop=(i == 7),
                )

            def evac(ps, bias_col, dst, n_cols, engine, name):
                """PSUM -> SBUF with fused per-o bias, then store."""
                pool = outp if n_cols == c_main else goutp
                ot = pool.tile([P, n_cols], f32, tag=f"ot{n_cols}", name=name)
                if engine == 0:
                    nc.scalar.activation(
                        ot[:],
                        ps[:],
                        mybir.ActivationFunctionType.Identity,
                        bias=bias_col,
                    )
                else:
                    nc.vector.tensor_scalar_add(ot[:], ps[:], bias_col)
                nc.scalar.dma_start(dst, ot[:])

            # Group 0 (o-blocks 0..GRP-1) runs k-major so every arriving
            # X-slab immediately feeds GRP o-columns of PE work.
            g0 = list(range(GRP))
            pss0 = {
                oi: psump.tile([P, c_main], f32, tag=f"ps{oi}", name=f"ps{oi}")
                for oi in g0
            }
            for k in range(kb):
                xvk = xqs[k // XQ]
                for oi in g0:
                    nc.tensor.matmul(
                        pss0[oi][:],
                        w_pref[oi][:, k, :],
                        xvk[:, k % XQ, :],
                        start=(k == 0),
                        stop=(k == kb - 1),
                    )
            for oi in g0:
                evac(
                    pss0[oi], bias_sb[:, oi : oi + 1],
                    ym[oi * P : (oi + 1) * P, :], c_main, oi % 2, f"ot{oi}",
                )

            # Remaining o-blocks run one at a time: per-bank k-runs rotate
            # through the PSUM banks (released by group 0 in the same
            # order), and evacuations pipeline under the next bank's MMs.
            for o in range(GRP, ob):
                w_o = load_w(o)
                ps = psump.tile([P, c_main], f32, tag=f"ps{o % GRP}", name=f"ps{o}")
                for k in range(kb):
                    nc.tensor.matmul(
                        ps[:],
                        w_o[:, k, :],
                        xqs[k // XQ][:, k % XQ, :],
                        start=(k == 0),
                        stop=(k == kb - 1),
                    )
                evac(
                    ps, bias_sb[:, o : o + 1],
                    ym[o * P : (o + 1) * P, :], c_main, o % 2, f"ot{o}",
                )

            # Guest pass: s_guest items of (GT tokens x oh o-blocks), final
            # outputs for their rectangle (no cross-core reduction).
            if s_guest:
                for s in range(s_guest):
                    for o in range(oh):
                        wg_o = wgp.tile(
                            [P, kb, P], bf16, tag="wg", name=f"wg{s}_{o}"
                        )
                        nc.sync.dma_start(wg_o[:], wg[s, o])
                        ps = psump.tile(
                            [P, GT], f32, tag=f"psg{o % 2}", name=f"psg{s}_{o}"
                        )
                        for k in range(kb):
                            nc.tensor.matmul(
                                ps[:],
                                wg_o[:, k, :],
                                xg_sb[s][:, k, :],
                                start=(k == 0),
                                stop=(k == kb - 1),
                            )
                        evac(
                            ps, biasg_sb[:, s * oh + o : s * oh + o + 1],
                            yg[s, o * P : (o + 1) * P, :], GT, o % 2,
                            f"go{s}_{o}",
                        )
    nc.finalize()
    return nc


def _tile_x(x_cols: np.ndarray, n_in: int, width: int) -> np.ndarray:
    """[n, n_in] fp32 token rows -> [P, kb, width] bf16 X^T tiling."""
    kb = n_in // P
    xt = np.zeros((n_in, width), np.float32)
    xt[:, : x_cols.shape[0]] = x_cols.T
    return np.ascontiguousarray(
        xt.reshape(kb, P, width).transpose(1, 0, 2)
    ).astype(ml_dtypes.bfloat16)


def _tile_w(w: np.ndarray) -> np.ndarray:
    """[n_in, n_o] fp32 -> [n_o//P, P, n_in//P, P] bf16 o-slab tiling."""
    n_in, n_o = w.shape
    return np.ascontiguousarray(
        w.reshape(n_in // P, P, n_o // P, P).transpose(2, 1, 0, 3)
    ).astype(ml_dtypes.bfloat16)


def _prepare(inputs, weight, bias, group_sizes):
    """Build (or reuse) the program and the per-core input maps."""
    inputs = np.ascontiguousarray(np.asarray(inputs, dtype=np.float32))
    weight = np.ascontiguousarray(np.asarray(weight, dtype=np.float32))
    bias = np.ascontiguousarray(np.asarray(bias, dtype=np.float32))
    g = np.asarray(group_sizes).astype(np.int64)

    t_tokens, n_in = inputs.shape
    n_exp, _, n_out = weight.shape
    assert n_exp == N_CORES, f"expected {N_CORES} experts, got {n_exp}"
    offs = np.concatenate([[0], np.cumsum(g)])
    assert offs[-1] == t_tokens, "group_sizes must sum to token count"

    kb, ob, oh = n_in // P, n_out // P, n_out // P // 2
    c_main = max(P, -(-t_tokens // N_CORES // P) * P)  # balanced share

    # Guest items: (expert, token-block, out-half) for tokens beyond c_main.
    items = []
    for e in range(n_exp):
        nblk = -(-max(0, int(g[e]) - c_main) // GT)
        for b in range(nblk):
            for h in range(2):
                items.append((e, b, h))
    s_guest = -(-len(items) // N_CORES) if items else 0

    key = (c_main, n_in, n_out, s_guest)
    if key not in _BUILD_CACHE:
        _BUILD_CACHE[key] = _build_program(c_main, n_in, n_out, s_guest)
    nc = _BUILD_CACHE[key]

    biasm_host = np.ascontiguousarray(bias.reshape(ob, P).T)  # [P, ob]

    # slot-major assignment: item i -> (core i % 8, slot i // 8)
    plan = [[None] * s_guest for _ in range(n_exp)]
    for i, it in enumerate(items):
        plan[i % N_CORES][i // N_CORES] = it

    in_maps = []
    for e in range(n_exp):
        nm = min(int(g[e]), c_main)
        m = {
            "xm": _tile_x(inputs[offs[e] : offs[e] + nm], n_in, c_main),
            "wm": _tile_w(weight[e]),
            "biasm": biasm_host,
        }
        if s_guest:
            xg_l, wg_l, bg_l = [], [], []
            for it in plan[e]:
                if it is None:
                    xg_l.append(np.zeros((P, kb, GT), ml_dtypes.bfloat16))
                    wg_l.append(np.zeros((oh, P, kb, P), ml_dtypes.bfloat16))
                    bg_l.append(np.zeros((P, oh), np.float32))
                else:
                    ge, b, h = it
                    t0 = offs[ge] + c_main + b * GT
                    n = min(GT, offs[ge + 1] - t0)
                    xg_l.append(_tile_x(inputs[t0 : t0 + n], n_in, GT))
                    wg_l.append(
                        _tile_w(weight[ge][:, h * oh * P : (h + 1) * oh * P])
                    )
                    bg_l.append(
                        np.ascontiguousarray(
                            bias.reshape(ob, P)[h * oh : (h + 1) * oh].T
                        )
                    )
            m["xg"] = np.stack(xg_l)
            m["wg"] = np.stack(wg_l)
            m["biasg"] = np.stack(bg_l)
        in_maps.append(m)
    meta = (g, offs, plan, c_main, s_guest)
    return nc, in_maps, meta, None, (t_tokens, n_out)


def kernel(inputs, weight, bias, group_sizes):
    nc, in_maps, meta, _, (t_tokens, n_out) = _prepare(
        inputs, weight, bias, group_sizes
    )
    g, offs, plan, c_main, s_guest = meta
    oh = n_out // P // 2
    res = run_bass_kernel_spmd(nc, in_maps, core_ids=list(range(N_CORES)))

    out = np.empty((t_tokens, n_out), np.float32)
    for e in range(N_CORES):
        nm = min(int(g[e]), c_main)
        if nm:
            out[offs[e] : offs[e] + nm] = res.results[e]["ym"][:, :nm].T
        for s in range(s_guest):
            it = plan[e][s]
            if it is None:
                continue
            ge, b, h = it
            t0 = offs[ge] + c_main + b * GT
            n = min(GT, int(offs[ge + 1] - t0))
            out[t0 : t0 + n, h * oh * P : (h + 1) * oh * P] = res.results[e][
                "yg"
            ][s][:, :n].T
    return out
